# revision 1
# baseline (speedup 1.0000x reference)
"""Block-Circulant-Matrix Linear kernel for Trainium2 (8 NeuronCores, SPMD).

Reference computation:
    W[r*64+i, q*64+j] = w[r, q, (i-j) % 64]        (dense 1024x1024 from w[16,16,64])
    y = x @ W.T                                    (x: [32768, 1024] f32)

Strategy (data-parallel, per sharding hint):
  - Shard x along tokens across 8 cores (4096 tokens each); replicate w.
  - Per core, y_tile = x_tile @ W.T via TensorE with fp32r (full-rate, reduced
    mantissa) matmuls:
      * lhsT = x-tile transposed on TensorE (PE transpose), rounded to fp32r by
        the ScalarE PSUM->SBUF copy.
      * rhs = the circulant W.T is never materialized.  Instead each in-channel
        chunk c keeps a "skewed" SBUF tile S_c[p=(qh,j), f] = w2r2[(2c+qh)*2048
        + f + j], where w2r2[q, r, t'] = w[r, q, (63-t') % 64] is a reversed,
        doubled, (q,r)-transposed copy of w staged in DRAM.  The skew (+j per
        partition) is free in the DMA (partition step 1 over DRAM), and a
        strided rhs access pattern [(rr: 128), (ii: 1)] then reads
          S_c[(qh,j), n*1024 + rr*128 + ii] = w[r, 2c+qh, (63-ii-j) % 64]
        which is exactly W.T with each 64-block of the out-dim reversed
        (ii = 63-i).  The reversal is undone for free by a negative-step AP in
        the VectorE PSUM->SBUF copy of y.
  - All DMAs use large contiguous descriptors; no slow gather anywhere.
"""

import numpy as np

N_CORES = 8
N_TOKENS = 32768
TOK_PER_CORE = N_TOKENS // N_CORES  # 4096
IN_CH = 1024
OUT_CH = 1024
BS = 64
R = OUT_CH // BS  # 16
Q = IN_CH // BS   # 16
KCH = IN_CH // 128  # 8 k-chunks of 128 partitions
S_FREE = (R - 1) * 2 * BS + BS  # 1984: covers max n*1024 + rr*128 + ii (+j via skew)

_CACHE = {}


def build_nc(tok_per_core=TOK_PER_CORE):
    from contextlib import ExitStack

    import concourse.bass as bass
    import concourse.mybir as mybir
    import concourse.tile as tile
    from concourse import bacc
    from concourse.masks import make_identity

    f32 = mybir.dt.float32
    f32r = mybir.dt.float32r

    nc = bacc.Bacc("TRN2", target_bir_lowering=False, debug=False)
    x = nc.dram_tensor("x", [tok_per_core, IN_CH], f32, kind="ExternalInput").ap()
    w = nc.dram_tensor("w", [R, Q, BS], f32, kind="ExternalInput").ap()
    y = nc.dram_tensor("y", [tok_per_core, OUT_CH], f32, kind="ExternalOutput").ap()

    n_tok_tiles = tok_per_core // 128

    def rev_last(ap3):
        """Reverse the last (innermost free) dim of an AP."""
        pairs = [list(p) for p in ap3.ap]
        n = pairs[-1][1]
        assert pairs[-1][0] == 1
        pairs[-1][0] = -1
        return bass.AP(ap3.tensor, ap3.offset + n - 1, pairs)

    with tile.TileContext(nc) as tc, ExitStack() as ctx:
        const_pool = ctx.enter_context(tc.tile_pool(name="const", bufs=1))
        s_pool = ctx.enter_context(tc.tile_pool(name="s", bufs=1))
        dram_pool = ctx.enter_context(tc.tile_pool(name="dram", bufs=1, space="DRAM"))
        xb_pool = ctx.enter_context(tc.tile_pool(name="xb", bufs=6))
        xt_sb_pool = ctx.enter_context(tc.tile_pool(name="xt_sb", bufs=10))
        y_sb_pool = ctx.enter_context(tc.tile_pool(name="y_sb", bufs=4))
        xt_ps_pool = ctx.enter_context(tc.tile_pool(name="xt_ps", bufs=2, space="PSUM"))
        y_ps_pool = ctx.enter_context(tc.tile_pool(name="y_ps", bufs=2, space="PSUM"))

        identity = const_pool.tile([128, 128], f32)
        make_identity(nc, identity)

        # --- stage w2r2[q, r, t'] = w[r, q, (63-t') % 64] in DRAM (f32r) ---
        # w flat is [(r q) = 256, 64]; two SBUF tiles of [128, 64] (r in [8a, 8a+8)).
        # The (r,q)->(q,r) reorder and the doubling are fused into the
        # SBUF->DRAM store: dst walks (r_local, q, s) to match the source
        # partition order.
        w_flat = w.rearrange("r q s -> (r q) s")
        w2r2 = dram_pool.tile([Q, R, 2 * BS], f32r)
        with tc.high_priority():
            for a in range(2):
                w_sb = const_pool.tile([128, BS], f32, name=f"w_sb_{a}")
                nc.sync.dma_start(w_sb, w_flat[a * 128 : (a + 1) * 128, :])
                w_rev = const_pool.tile([128, BS], f32r, name=f"w_rev_{a}")
                nc.vector.tensor_copy(w_rev, rev_last(w_sb[:, :]))
                for half in range(2):
                    dst3 = bass.AP(
                        w2r2.tensor,
                        w2r2.offset + a * (R // 2) * 2 * BS + half * BS,
                        [[2 * BS, R // 2], [R * 2 * BS, Q], [1, BS]],
                    )
                    nc.sync.dma_start(dst3, w_rev[:, :])

        # --- skewed replica tiles S_c[(qh,j), f] = w2r2_flat[(2c+qh)*2048 + f + j] ---
        # DMAs are emitted interleaved with the first token tiles (see loop) so
        # the scheduler staggers them against x-loads and transposes.
        s_tiles = [s_pool.tile([128, S_FREE], f32r, name=f"s_{c}") for c in range(KCH)]

        def emit_s_dma(c):
            s_c = s_tiles[c]
            for qh in range(2):
                src = bass.AP(
                    w2r2.tensor,
                    w2r2.offset + (2 * c + qh) * R * 2 * BS,
                    [[1, BS], [1, S_FREE]],
                )
                eng = nc.scalar if qh == 0 else nc.sync
                eng.dma_start(s_c[qh * BS : (qh + 1) * BS, :], src)

        def rhs_ap(c, n):
            s_c = s_tiles[c]
            pstride = s_c[:, :].ap[0][0]
            return bass.AP(
                s_c.tensor,
                s_c.offset + n * (R // 2) * 2 * BS,
                [[pstride, 128], [2 * BS, R // 2], [1, BS]],
            )

        # --- main loop over 128-token tiles, software-pipelined by one tile:
        # transposes + PSUM->SBUF rounding copies for tile t are emitted before
        # the matmuls of tile t-1 so the PE never waits on the ScalarE copy.
        xts = {}

        def emit_front(t):
            xb = xb_pool.tile([128, IN_CH], f32, name=f"xb_{t}", tag="xb")
            # ramp tiles ride SWDGE so both HWDGE queues are dedicated to the
            # skewed-weight stream (the binding startup constraint)
            xb_eng = nc.gpsimd if t < 8 else nc.sync
            xb_eng.dma_start(xb, x[t * 128 : (t + 1) * 128, :])
            xt_ps = xt_ps_pool.tile([128, IN_CH], f32, name=f"xt_ps_{t}", tag="xt_ps")
            for c in range(KCH):
                nc.tensor.transpose(
                    xt_ps[:, c * 128 : (c + 1) * 128],
                    xb[:, c * 128 : (c + 1) * 128],
                    identity,
                )
            xt = xt_sb_pool.tile([128, IN_CH], f32r, name=f"xt_{t}", tag="xt")
            nc.scalar.copy(xt[:, 0:512], xt_ps[:, 0:512])
            nc.scalar.copy(xt[:, 512:1024], xt_ps[:, 512:1024])
            xts[t] = xt

        def emit_back(t):
            xt = xts.pop(t)
            y_ps = y_ps_pool.tile([128, OUT_CH], f32, name=f"y_ps_{t}", tag="y_ps")
            for c in range(KCH):
                for n in range(OUT_CH // 512):
                    nc.tensor.matmul(
                        y_ps[:, n * 512 : (n + 1) * 512],
                        lhsT=xt[:, c * 128 : (c + 1) * 128],
                        rhs=rhs_ap(c, n),
                        start=(c == 0),
                        stop=(c == KCH - 1),
                    )
            # copy PSUM->SBUF while un-reversing each 64-block of the out-dim:
            #   y_sb[p, n*512 + rr*64 + (63-ii)] = y_ps[p, n*512 + rr*64 + ii]
            y_sb = y_sb_pool.tile([128, OUT_CH], f32, name=f"y_sb_{t}", tag="y_sb")
            for n in range(2):
                src = y_ps[:, n * 512 : (n + 1) * 512].rearrange(
                    "p (r i) -> p r i", i=BS
                )
                dst = rev_last(
                    y_sb[:, n * 512 : (n + 1) * 512].rearrange("p (r i) -> p r i", i=BS)
                )
                nc.vector.tensor_copy(dst, src)
            nc.sync.dma_start(y[t * 128 : (t + 1) * 128, :], y_sb)

        # pipeline depth: all S-chunk DMAs are emitted during the first DEPTH
        # fronts (program order requires every S write before the first matmul
        # emission), and matmuls trail the transposes by DEPTH tiles.
        depth = min(KCH, n_tok_tiles)
        for c in range(depth, KCH):
            emit_s_dma(c)
        for t in range(n_tok_tiles + depth):
            if t < depth:
                emit_s_dma(t)
            if t < n_tok_tiles:
                emit_front(t)
            if t >= depth:
                emit_back(t - depth)

    nc.compile()
    return nc


def get_nc(tok_per_core=TOK_PER_CORE):
    if tok_per_core not in _CACHE:
        _CACHE[tok_per_core] = build_nc(tok_per_core)
    return _CACHE[tok_per_core]


def kernel(x: np.ndarray, w: np.ndarray) -> np.ndarray:
    from concourse.bass_utils import run_bass_kernel_spmd

    x = np.ascontiguousarray(x, dtype=np.float32)
    w = np.ascontiguousarray(w, dtype=np.float32)
    assert x.shape == (N_TOKENS, IN_CH), x.shape
    assert w.shape == (R, Q, BS), w.shape

    nc = get_nc()
    in_maps = [
        {"x": x[i * TOK_PER_CORE : (i + 1) * TOK_PER_CORE], "w": w}
        for i in range(N_CORES)
    ]
    res = run_bass_kernel_spmd(nc, in_maps, core_ids=list(range(N_CORES)))
    return np.concatenate([r["y"] for r in res.results], axis=0)



# revision 3
# speedup vs baseline: 1.0118x; 1.0118x over previous
"""Block-Circulant-Matrix Linear kernel for Trainium2 (8 NeuronCores, SPMD).

Reference computation:
    W[r*64+i, q*64+j] = w[r, q, (i-j) % 64]        (dense 1024x1024 from w[16,16,64])
    y = x @ W.T                                    (x: [32768, 1024] f32)

Strategy (data-parallel, per sharding hint):
  - Shard x along tokens across 8 cores (4096 tokens each); replicate w.
  - Per core, y_tile = x_tile @ W.T via TensorE in bf16 (full-rate streams,
    FWL weight loads, half-cost transposes vs fp32):
      * x is cast f32->bf16 on ScalarE, then transposed per 128-chunk on
        TensorE (bf16 transpose = 1 cycle/row vs 2 for f32); the bf16 PSUM
        transpose tile is copied to SBUF by VectorE at 2x (2-byte) rate.
      * rhs = the circulant W.T is never materialized.  Each in-channel
        chunk c keeps a "skewed" SBUF tile S_c[p=(qh,j), f] = w2r2[(2c+qh)*2048
        + f + j], where w2r2[q, r, t'] = w[r, q, (63-t') % 64] is a reversed,
        doubled, (q,r)-transposed bf16 copy of w staged in DRAM.  The skew
        (+j per partition) is free in the DMA (partition step 1 over DRAM),
        and a strided rhs access pattern [(rr: 128), (ii: 1)] then reads
        exactly W.T with each 64-block of the out-dim reversed (ii = 63-i).
        The reversal is undone for free by a negative-step AP in the
        VectorE PSUM->SBUF copy of y.
  - y is stored to DRAM in bf16 (halves store traffic; ~2e-3 max rel err,
    well within the 2e-2 gate) and upcast to f32 on the host.
  - All DMAs use large contiguous descriptors; no slow gather anywhere.
"""

import numpy as np

N_CORES = 8
N_TOKENS = 32768
TOK_PER_CORE = N_TOKENS // N_CORES  # 4096
IN_CH = 1024
OUT_CH = 1024
BS = 64
R = OUT_CH // BS  # 16
Q = IN_CH // BS   # 16
KCH = IN_CH // 128  # 8 k-chunks of 128 partitions
S_FREE = (R - 1) * 2 * BS + BS  # 1984: covers max n*1024 + rr*128 + ii (+j via skew)

_CACHE = {}


def build_nc(tok_per_core=TOK_PER_CORE):
    from contextlib import ExitStack

    import concourse.bass as bass
    import concourse.mybir as mybir
    import concourse.tile as tile
    from concourse import bacc
    from concourse.masks import make_identity

    f32 = mybir.dt.float32
    bf16 = mybir.dt.bfloat16

    nc = bacc.Bacc("TRN2", target_bir_lowering=False, debug=False)
    x = nc.dram_tensor("x", [tok_per_core, IN_CH], f32, kind="ExternalInput").ap()
    w = nc.dram_tensor("w", [R, Q, BS], f32, kind="ExternalInput").ap()
    y = nc.dram_tensor("y", [tok_per_core, OUT_CH], bf16, kind="ExternalOutput").ap()

    n_tok_tiles = tok_per_core // 128

    def rev_last(ap3):
        """Reverse the last (innermost free) dim of an AP."""
        pairs = [list(p) for p in ap3.ap]
        n = pairs[-1][1]
        assert pairs[-1][0] == 1
        pairs[-1][0] = -1
        return bass.AP(ap3.tensor, ap3.offset + n - 1, pairs)

    with tile.TileContext(nc) as tc, ExitStack() as ctx:
        const_pool = ctx.enter_context(tc.tile_pool(name="const", bufs=1))
        s_pool = ctx.enter_context(tc.tile_pool(name="s", bufs=1))
        dram_pool = ctx.enter_context(tc.tile_pool(name="dram", bufs=1, space="DRAM"))
        xb_pool = ctx.enter_context(tc.tile_pool(name="xb", bufs=6))
        xh_pool = ctx.enter_context(tc.tile_pool(name="xh", bufs=4))
        xt_sb_pool = ctx.enter_context(tc.tile_pool(name="xt_sb", bufs=10))
        y_sb_pool = ctx.enter_context(tc.tile_pool(name="y_sb", bufs=4))
        xt_ps_pool = ctx.enter_context(tc.tile_pool(name="xt_ps", bufs=2, space="PSUM"))
        y_ps_pool = ctx.enter_context(tc.tile_pool(name="y_ps", bufs=2, space="PSUM"))

        identity = const_pool.tile([128, 128], bf16)
        make_identity(nc, identity)

        # --- stage w2r2[q, r, t'] = w[r, q, (63-t') % 64] in DRAM (bf16) ---
        # w flat is [(r q) = 256, 64]; two SBUF tiles of [128, 64] (r in [8a, 8a+8)).
        # The (r,q)->(q,r) reorder and the doubling are fused into the
        # SBUF->DRAM store: dst walks (r_local, q, s) to match the source
        # partition order.
        w_flat = w.rearrange("r q s -> (r q) s")
        w2r2 = dram_pool.tile([Q, R, 2 * BS], bf16)
        with tc.high_priority():
            for a in range(2):
                w_sb = const_pool.tile([128, BS], f32, name=f"w_sb_{a}")
                nc.sync.dma_start(w_sb, w_flat[a * 128 : (a + 1) * 128, :])
                w_rev = const_pool.tile([128, BS], bf16, name=f"w_rev_{a}")
                nc.vector.tensor_copy(w_rev, rev_last(w_sb[:, :]))
                for half in range(2):
                    dst3 = bass.AP(
                        w2r2.tensor,
                        w2r2.offset + a * (R // 2) * 2 * BS + half * BS,
                        [[2 * BS, R // 2], [R * 2 * BS, Q], [1, BS]],
                    )
                    nc.sync.dma_start(dst3, w_rev[:, :])

        # --- skewed replica tiles S_c[(qh,j), f] = w2r2_flat[(2c+qh)*2048 + f + j] ---
        # DMAs are emitted interleaved with the first token tiles (see loop) so
        # the scheduler staggers them against x-loads and transposes.
        s_tiles = [s_pool.tile([128, S_FREE], bf16, name=f"s_{c}") for c in range(KCH)]

        def emit_s_dma(c):
            s_c = s_tiles[c]
            for qh in range(2):
                src = bass.AP(
                    w2r2.tensor,
                    w2r2.offset + (2 * c + qh) * R * 2 * BS,
                    [[1, BS], [1, S_FREE]],
                )
                eng = nc.scalar if qh == 0 else nc.sync
                eng.dma_start(s_c[qh * BS : (qh + 1) * BS, :], src)

        def rhs_ap(c, n):
            s_c = s_tiles[c]
            pstride = s_c[:, :].ap[0][0]
            return bass.AP(
                s_c.tensor,
                s_c.offset + n * (R // 2) * 2 * BS,
                [[pstride, 128], [2 * BS, R // 2], [1, BS]],
            )

        # --- main loop over 128-token tiles, software-pipelined: transposes +
        # PSUM->SBUF copies for tile t are emitted before the matmuls of tile
        # t-depth so the PE never waits on the ScalarE/VectorE copies.
        xts = {}

        def emit_front(t):
            xb = xb_pool.tile([128, IN_CH], f32, name=f"xb_{t}", tag="xb")
            # ramp tiles ride SWDGE so both HWDGE queues are dedicated to the
            # skewed-weight stream (the binding startup constraint)
            xb_eng = nc.gpsimd if t < 8 else nc.sync
            xb_eng.dma_start(xb, x[t * 128 : (t + 1) * 128, :])
            # cast to bf16 on ScalarE so the PE transposes at 1 cycle/row
            xh = xh_pool.tile([128, IN_CH], bf16, name=f"xh_{t}", tag="xh")
            nc.scalar.copy(xh, xb)
            xt_ps = xt_ps_pool.tile([128, IN_CH], bf16, name=f"xt_ps_{t}", tag="xt_ps")
            for c in range(KCH):
                nc.tensor.transpose(
                    xt_ps[:, c * 128 : (c + 1) * 128],
                    xh[:, c * 128 : (c + 1) * 128],
                    identity,
                )
            xt = xt_sb_pool.tile([128, IN_CH], bf16, name=f"xt_{t}", tag="xt")
            nc.vector.tensor_copy(xt, xt_ps)
            xts[t] = xt

        def emit_back(t):
            xt = xts.pop(t)
            y_ps = y_ps_pool.tile([128, OUT_CH], f32, name=f"y_ps_{t}", tag="y_ps")
            for c in range(KCH):
                for n in range(OUT_CH // 512):
                    nc.tensor.matmul(
                        y_ps[:, n * 512 : (n + 1) * 512],
                        lhsT=xt[:, c * 128 : (c + 1) * 128],
                        rhs=rhs_ap(c, n),
                        start=(c == 0),
                        stop=(c == KCH - 1),
                    )
            # copy PSUM->SBUF while un-reversing each 64-block of the out-dim:
            #   y_sb[p, n*512 + rr*64 + (63-ii)] = y_ps[p, n*512 + rr*64 + ii]
            # split across ScalarE (n=0) and VectorE (n=1) to balance engines
            y_sb = y_sb_pool.tile([128, OUT_CH], bf16, name=f"y_sb_{t}", tag="y_sb")
            for n in range(2):
                src = y_ps[:, n * 512 : (n + 1) * 512].rearrange(
                    "p (r i) -> p r i", i=BS
                )
                dst = rev_last(
                    y_sb[:, n * 512 : (n + 1) * 512].rearrange("p (r i) -> p r i", i=BS)
                )
                if n == 0:
                    nc.scalar.copy(dst, src)
                else:
                    nc.vector.tensor_copy(dst, src)
            nc.sync.dma_start(y[t * 128 : (t + 1) * 128, :], y_sb)

        # pipeline depth: all S-chunk DMAs are emitted during the first DEPTH
        # fronts (program order requires every S write before the first matmul
        # emission), and matmuls trail the transposes by DEPTH tiles.
        depth = min(KCH, n_tok_tiles)
        for c in range(depth, KCH):
            emit_s_dma(c)
        for t in range(n_tok_tiles + depth):
            if t < depth:
                emit_s_dma(t)
            if t < n_tok_tiles:
                emit_front(t)
            if t >= depth:
                emit_back(t - depth)

    nc.compile()
    return nc


def get_nc(tok_per_core=TOK_PER_CORE):
    if tok_per_core not in _CACHE:
        _CACHE[tok_per_core] = build_nc(tok_per_core)
    return _CACHE[tok_per_core]


def kernel(x: np.ndarray, w: np.ndarray) -> np.ndarray:
    from concourse.bass_utils import run_bass_kernel_spmd

    x = np.ascontiguousarray(x, dtype=np.float32)
    w = np.ascontiguousarray(w, dtype=np.float32)
    assert x.shape == (N_TOKENS, IN_CH), x.shape
    assert w.shape == (R, Q, BS), w.shape

    nc = get_nc()
    in_maps = [
        {"x": x[i * TOK_PER_CORE : (i + 1) * TOK_PER_CORE], "w": w}
        for i in range(N_CORES)
    ]
    res = run_bass_kernel_spmd(nc, in_maps, core_ids=list(range(N_CORES)))
    return np.concatenate(
        [np.asarray(r["y"]).astype(np.float32) for r in res.results], axis=0
    )


# revision 4
# speedup vs baseline: 1.2802x; 1.2653x over previous
"""Block-Circulant-Matrix Linear kernel for Trainium2 (8 NeuronCores, SPMD).

Reference computation:
    W[r*64+i, q*64+j] = w[r, q, (i-j) % 64]        (dense 1024x1024 from w[16,16,64])
    y = x @ W.T                                    (x: [32768, 1024] f32)

Strategy (data-parallel, per sharding hint):
  - Shard x along tokens across 8 cores (4096 tokens each); replicate the
    weight.  The dense W.T (the reference materializes exactly this) is built
    once on the host from the 64 KB compressed w and passed in as a 2 MB bf16
    tensor -- cheaper to DMA than an on-device skewed expansion, and it gives
    the TensorE a fully contiguous moving operand.
  - Per core, y_tile = x_tile @ W.T in bf16 on TensorE:
      * x is cast f32->bf16 on ScalarE, transposed per 128-chunk on TensorE
        (bf16 transpose = 1 cycle/row), and the bf16 PSUM transpose tile is
        copied to SBUF by VectorE at 2x 2-byte rate.
      * per token tile, the 8 transposes (for tile t) are interleaved between
        the 16 matmuls (for tile t-2) so every LDWEIGHTS hides under the
        previous matmul's 512-column stream.
      * a short warm-up spin of dummy transposes keeps the PE HAM clock-gate
        at 8/8 (2.4 GHz) through the DMA ramp, so the first real matmuls do
        not pay the 1.2 GHz cold window.
  - y is stored to DRAM in bf16 (halves store traffic; ~2e-3 max rel err,
    well within the 2e-2 gate) and upcast to f32 on the host.
  - All DMAs use large contiguous descriptors; no gather anywhere.
"""

import numpy as np

N_CORES = 8
N_TOKENS = 32768
TOK_PER_CORE = N_TOKENS // N_CORES  # 4096
IN_CH = 1024
OUT_CH = 1024
BS = 64
R = OUT_CH // BS  # 16
Q = IN_CH // BS   # 16
KCH = IN_CH // 128  # 8 k-chunks of 128 partitions
N_WARMUP = 32      # dummy transposes to keep the HAM clock-gate warm
DEPTH = 2          # matmuls trail transposes by DEPTH token tiles
XB_LEAD = 3        # x-tile DMA prefetch distance

_CACHE = {}


def build_nc(tok_per_core=TOK_PER_CORE):
    from contextlib import ExitStack

    import concourse.mybir as mybir
    import concourse.tile as tile
    from concourse import bacc
    from concourse.masks import make_identity

    f32 = mybir.dt.float32
    bf16 = mybir.dt.bfloat16

    nc = bacc.Bacc("TRN2", target_bir_lowering=False, debug=False)
    x = nc.dram_tensor("x", [tok_per_core, IN_CH], f32, kind="ExternalInput").ap()
    wt = nc.dram_tensor("wt", [IN_CH, OUT_CH], bf16, kind="ExternalInput").ap()
    y = nc.dram_tensor("y", [tok_per_core, OUT_CH], bf16, kind="ExternalOutput").ap()

    n = tok_per_core // 128  # token tiles

    with tile.TileContext(nc) as tc, ExitStack() as ctx:
        const_pool = ctx.enter_context(tc.tile_pool(name="const", bufs=1))
        wt_pool = ctx.enter_context(tc.tile_pool(name="wt", bufs=1))
        xb_pool = ctx.enter_context(tc.tile_pool(name="xb", bufs=6))
        xh_pool = ctx.enter_context(tc.tile_pool(name="xh", bufs=4))
        xt_sb_pool = ctx.enter_context(tc.tile_pool(name="xt_sb", bufs=4))
        y_sb_pool = ctx.enter_context(tc.tile_pool(name="y_sb", bufs=4))
        xt_ps_pool = ctx.enter_context(tc.tile_pool(name="xt_ps", bufs=2, space="PSUM"))
        y_ps_pool = ctx.enter_context(tc.tile_pool(name="y_ps", bufs=2, space="PSUM"))
        warm_pool = ctx.enter_context(tc.tile_pool(name="warm", bufs=1, space="PSUM"))

        identity = const_pool.tile([128, 128], bf16)
        make_identity(nc, identity)

        # --- PE warm-up: dummy transposes trip the HAM SHORT window (~3.4us
        # of sustained activity) so the real matmuls start at 2.4 GHz.  The
        # scratch PSUM tile is never read.
        warm_ps = warm_pool.tile([128, 128], bf16)
        for _ in range(N_WARMUP):
            nc.tensor.transpose(warm_ps, identity, identity)

        # --- dense W.T chunks: wt_sb[c][p, o] = wt[c*128 + p, o] (bf16) ---
        wt_sb = [wt_pool.tile([128, OUT_CH], bf16, name=f"wt_{c}") for c in range(KCH)]
        for c in range(KCH):
            nc.scalar.dma_start(wt_sb[c], wt[c * 128 : (c + 1) * 128, :])

        xbs, xhs, xts = {}, {}, {}

        def emit_xb(t):
            xb = xb_pool.tile([128, IN_CH], f32, name=f"xb_{t}", tag="xb")
            xb_eng = nc.gpsimd if t < 4 else nc.sync
            xb_eng.dma_start(xb, x[t * 128 : (t + 1) * 128, :])
            xbs[t] = xb

        def emit_cast(t):
            xh = xh_pool.tile([128, IN_CH], bf16, name=f"xh_{t}", tag="xh")
            nc.scalar.copy(xh, xbs.pop(t))
            xhs[t] = xh

        for t in range(min(XB_LEAD, n)):
            emit_xb(t)
        emit_cast(0)

        for s in range(n + DEPTH):
            if s + XB_LEAD < n:
                emit_xb(s + XB_LEAD)
            if s + 1 < n:
                emit_cast(s + 1)
            if s < n:
                xh = xhs.pop(s)
                xt_ps = xt_ps_pool.tile(
                    [128, IN_CH], bf16, name=f"xt_ps_{s}", tag="xt_ps"
                )
            if s >= DEPTH:
                xt = xts.pop(s - DEPTH)
                y_ps = y_ps_pool.tile(
                    [128, OUT_CH], f32, name=f"y_ps_{s - DEPTH}", tag="y_ps"
                )
            # interleave tile s's transposes between tile (s-DEPTH)'s matmul
            # pairs: every LDWEIGHTS hides under the previous 512-col stream
            for c in range(KCH):
                if s < n:
                    nc.tensor.transpose(
                        xt_ps[:, c * 128 : (c + 1) * 128],
                        xh[:, c * 128 : (c + 1) * 128],
                        identity,
                    )
                if s >= DEPTH:
                    for half in range(2):
                        nc.tensor.matmul(
                            y_ps[:, half * 512 : (half + 1) * 512],
                            lhsT=xt[:, c * 128 : (c + 1) * 128],
                            rhs=wt_sb[c][:, half * 512 : (half + 1) * 512],
                            start=(c == 0),
                            stop=(c == KCH - 1),
                        )
            if s < n:
                xt_new = xt_sb_pool.tile([128, IN_CH], bf16, name=f"xt_{s}", tag="xt")
                nc.vector.tensor_copy(xt_new, xt_ps)
                xts[s] = xt_new
            if s >= DEPTH:
                t = s - DEPTH
                y_sb = y_sb_pool.tile([128, OUT_CH], bf16, name=f"y_sb_{t}", tag="y_sb")
                nc.scalar.copy(y_sb[:, 0:512], y_ps[:, 0:512])
                nc.vector.tensor_copy(y_sb[:, 512:1024], y_ps[:, 512:1024])
                nc.sync.dma_start(y[t * 128 : (t + 1) * 128, :], y_sb)

    nc.compile()
    return nc


def get_nc(tok_per_core=TOK_PER_CORE):
    if tok_per_core not in _CACHE:
        _CACHE[tok_per_core] = build_nc(tok_per_core)
    return _CACHE[tok_per_core]


def _build_wt_bf16(w: np.ndarray) -> np.ndarray:
    """Dense W.T [in, out] in bf16 from compressed w[R, Q, BS] (host side,
    same construction as the reference's _build_dense_weight)."""
    import ml_dtypes

    i = np.arange(BS)
    idx = (i[:, None] - i[None, :]) % BS          # (bs, bs) circulant index
    Wb = w[:, :, idx]                             # (R, Q, bs, bs)
    W = Wb.transpose(0, 2, 1, 3).reshape(OUT_CH, IN_CH)
    return np.ascontiguousarray(W.T).astype(ml_dtypes.bfloat16)


def kernel(x: np.ndarray, w: np.ndarray) -> np.ndarray:
    from concourse.bass_utils import run_bass_kernel_spmd

    x = np.ascontiguousarray(x, dtype=np.float32)
    w = np.ascontiguousarray(w, dtype=np.float32)
    assert x.shape == (N_TOKENS, IN_CH), x.shape
    assert w.shape == (R, Q, BS), w.shape

    wt = _build_wt_bf16(w)
    nc = get_nc()
    in_maps = [
        {"x": x[i * TOK_PER_CORE : (i + 1) * TOK_PER_CORE], "wt": wt}
        for i in range(N_CORES)
    ]
    res = run_bass_kernel_spmd(nc, in_maps, core_ids=list(range(N_CORES)))
    return np.concatenate(
        [np.asarray(r["y"]).astype(np.float32) for r in res.results], axis=0
    )


# revision 8
# speedup vs baseline: 1.3163x; 1.0282x over previous
"""Block-Circulant-Matrix Linear kernel for Trainium2 (8 NeuronCores, SPMD).

Reference computation:
    W[r*64+i, q*64+j] = w[r, q, (i-j) % 64]        (dense 1024x1024 from w[16,16,64])
    y = x @ W.T                                    (x: [32768, 1024] f32)

Strategy (data-parallel, per sharding hint):
  - Shard x along tokens across 8 cores (4096 tokens each); replicate the
    weight.  The dense W.T (the reference materializes exactly this) is built
    once on the host from the 64 KB compressed w and passed in as a 2 MB bf16
    tensor -- cheaper to DMA than an on-device skewed expansion, and it gives
    the TensorE a fully contiguous moving operand.
  - Per core, y_tile = x_tile @ W.T in bf16 on TensorE:
      * x is cast f32->bf16 on ScalarE, transposed per 128-chunk on TensorE
        (bf16 transpose = 1 cycle/row), and the bf16 PSUM transpose tile is
        copied to SBUF by VectorE at 2x 2-byte rate.
      * per token tile, the 8 transposes (for tile t) are interleaved between
        the 16 matmuls (for tile t-2) so every LDWEIGHTS hides under the
        previous matmul's 512-column stream.
      * a short warm-up spin of dummy transposes keeps the PE HAM clock-gate
        at 8/8 (2.4 GHz) through the DMA ramp, so the first real matmuls do
        not pay the 1.2 GHz cold window.
  - y is stored to DRAM in bf16 (halves store traffic; ~2e-3 max rel err,
    well within the 2e-2 gate) and upcast to f32 on the host.
  - All DMAs use large contiguous descriptors; no gather anywhere.
"""

import numpy as np

N_CORES = 8
N_TOKENS = 32768
TOK_PER_CORE = N_TOKENS // N_CORES  # 4096
IN_CH = 1024
OUT_CH = 1024
BS = 64
R = OUT_CH // BS  # 16
Q = IN_CH // BS   # 16
KCH = IN_CH // 128  # 8 k-chunks of 128 partitions
N_WARMUP = 88      # dummy transposes to keep the HAM clock-gate warm
DEPTH = 2          # matmuls trail transposes by DEPTH token tiles
XB_LEAD = 3        # x-tile DMA prefetch distance

_CACHE = {}


def build_nc(tok_per_core=TOK_PER_CORE):
    from contextlib import ExitStack

    import concourse.mybir as mybir
    import concourse.tile as tile
    from concourse import bacc
    from concourse.masks import make_identity

    f32 = mybir.dt.float32
    bf16 = mybir.dt.bfloat16

    nc = bacc.Bacc("TRN2", target_bir_lowering=False, debug=False)
    x = nc.dram_tensor("x", [tok_per_core, IN_CH], f32, kind="ExternalInput").ap()
    wt = nc.dram_tensor("wt", [IN_CH, OUT_CH], bf16, kind="ExternalInput").ap()
    y = nc.dram_tensor("y", [tok_per_core, OUT_CH], bf16, kind="ExternalOutput").ap()

    n = tok_per_core // 128  # token tiles

    with tile.TileContext(nc) as tc, ExitStack() as ctx:
        const_pool = ctx.enter_context(tc.tile_pool(name="const", bufs=1))
        wt_pool = ctx.enter_context(tc.tile_pool(name="wt", bufs=1))
        xb_pool = ctx.enter_context(tc.tile_pool(name="xb", bufs=6))
        xh_pool = ctx.enter_context(tc.tile_pool(name="xh", bufs=4))
        xt_sb_pool = ctx.enter_context(tc.tile_pool(name="xt_sb", bufs=4))
        y_sb_pool = ctx.enter_context(tc.tile_pool(name="y_sb", bufs=4))
        xt_ps_pool = ctx.enter_context(tc.tile_pool(name="xt_ps", bufs=2, space="PSUM"))
        y_ps_pool = ctx.enter_context(tc.tile_pool(name="y_ps", bufs=2, space="PSUM"))
        warm_pool = ctx.enter_context(tc.tile_pool(name="warm", bufs=1, space="PSUM"))

        identity = const_pool.tile([128, 128], bf16)
        make_identity(nc, identity)

        # --- PE warm-up: dummy transposes trip the HAM SHORT window (~3.4us
        # of sustained activity) so the real matmuls start at 2.4 GHz.  The
        # scratch PSUM tile is never read.
        warm_ps = warm_pool.tile([128, 128], bf16)
        for _ in range(N_WARMUP):
            nc.tensor.transpose(warm_ps, identity, identity)

        # --- dense W.T chunks: wt_sb[c][p, o] = wt[c*128 + p, o] (bf16) ---
        wt_sb = [wt_pool.tile([128, OUT_CH], bf16, name=f"wt_{c}") for c in range(KCH)]
        for c in range(KCH):
            nc.scalar.dma_start(wt_sb[c], wt[c * 128 : (c + 1) * 128, :])

        xbs, xhs, xts = {}, {}, {}

        def emit_xb(t):
            xb = xb_pool.tile([128, IN_CH], f32, name=f"xb_{t}", tag="xb")
            nc.sync.dma_start(xb, x[t * 128 : (t + 1) * 128, :])
            xbs[t] = xb

        def emit_cast(t):
            xh = xh_pool.tile([128, IN_CH], bf16, name=f"xh_{t}", tag="xh")
            nc.scalar.copy(xh, xbs.pop(t))
            xhs[t] = xh

        for t in range(min(XB_LEAD, n)):
            emit_xb(t)
        emit_cast(0)

        for s in range(n + DEPTH):
            if s + XB_LEAD < n:
                emit_xb(s + XB_LEAD)
            if s + 1 < n:
                emit_cast(s + 1)
            if s < n:
                xh = xhs.pop(s)
                xt_ps = xt_ps_pool.tile(
                    [128, IN_CH], bf16, name=f"xt_ps_{s}", tag="xt_ps"
                )
            if s >= DEPTH:
                xt = xts.pop(s - DEPTH)
                y_ps = y_ps_pool.tile(
                    [128, OUT_CH], f32, name=f"y_ps_{s - DEPTH}", tag="y_ps"
                )
            # interleave tile s's transposes between tile (s-DEPTH)'s matmul
            # pairs: every LDWEIGHTS hides under the previous 512-col stream
            for c in range(KCH):
                if s < n:
                    nc.tensor.transpose(
                        xt_ps[:, c * 128 : (c + 1) * 128],
                        xh[:, c * 128 : (c + 1) * 128],
                        identity,
                    )
                if s >= DEPTH:
                    for half in range(2):
                        nc.tensor.matmul(
                            y_ps[:, half * 512 : (half + 1) * 512],
                            lhsT=xt[:, c * 128 : (c + 1) * 128],
                            rhs=wt_sb[c][:, half * 512 : (half + 1) * 512],
                            start=(c == 0),
                            stop=(c == KCH - 1),
                        )
            if s < n:
                xt_new = xt_sb_pool.tile([128, IN_CH], bf16, name=f"xt_{s}", tag="xt")
                nc.vector.tensor_copy(xt_new, xt_ps)
                xts[s] = xt_new
            if s >= DEPTH:
                t = s - DEPTH
                y_sb = y_sb_pool.tile([128, OUT_CH], bf16, name=f"y_sb_{t}", tag="y_sb")
                nc.scalar.copy(y_sb[:, 0:512], y_ps[:, 0:512])
                nc.vector.tensor_copy(y_sb[:, 512:1024], y_ps[:, 512:1024])
                # y stores ride the scalar HWDGE queue (free after the wt
                # load) so they never sit behind x loads on the sync queue
                nc.scalar.dma_start(y[t * 128 : (t + 1) * 128, :], y_sb)

    nc.compile()
    return nc


def get_nc(tok_per_core=TOK_PER_CORE):
    if tok_per_core not in _CACHE:
        _CACHE[tok_per_core] = build_nc(tok_per_core)
    return _CACHE[tok_per_core]


def _build_wt_bf16(w: np.ndarray) -> np.ndarray:
    """Dense W.T [in, out] in bf16 from compressed w[R, Q, BS] (host side,
    same construction as the reference's _build_dense_weight)."""
    import ml_dtypes

    i = np.arange(BS)
    idx = (i[:, None] - i[None, :]) % BS          # (bs, bs) circulant index
    Wb = w[:, :, idx]                             # (R, Q, bs, bs)
    W = Wb.transpose(0, 2, 1, 3).reshape(OUT_CH, IN_CH)
    return np.ascontiguousarray(W.T).astype(ml_dtypes.bfloat16)


def kernel(x: np.ndarray, w: np.ndarray) -> np.ndarray:
    from concourse.bass_utils import run_bass_kernel_spmd

    x = np.ascontiguousarray(x, dtype=np.float32)
    w = np.ascontiguousarray(w, dtype=np.float32)
    assert x.shape == (N_TOKENS, IN_CH), x.shape
    assert w.shape == (R, Q, BS), w.shape

    wt = _build_wt_bf16(w)
    nc = get_nc()
    in_maps = [
        {"x": x[i * TOK_PER_CORE : (i + 1) * TOK_PER_CORE], "wt": wt}
        for i in range(N_CORES)
    ]
    res = run_bass_kernel_spmd(nc, in_maps, core_ids=list(range(N_CORES)))
    return np.concatenate(
        [np.asarray(r["y"]).astype(np.float32) for r in res.results], axis=0
    )
